# revision 26
# baseline (speedup 1.0000x reference)
"""MixtureOfDepth Trainium2 Bass kernel (single-core, host-side routing).

Router matvec, top-k threshold selection, token gather and scatter are
per-batch-row independent and tiny, so they run on the host with the exact
same jax CPU ops as the reference (bit-identical selection). The device
runs only the dense transformer block on the 511 selected tokens of each
batch (both batches sequentially on core 0 — the block is ~15 GFLOP/batch,
~0.5 ms/batch): pre-LN attention with RoPE (bf16 matmuls, f32 accum) and
the full-DFF MLP with W1/W2 streamed in 1024-column chunks.

The axon tunnel is latency-bound (~70-100 ms per round trip regardless of
size), so the per-call traffic is collapsed to ONE packed bf16 input
buffer (tokens + RoPE tables for both batches, ~2.2 MB) and ONE bf16
output buffer (~2 MB) on a single device via a persistent jit wrapper;
weights ship pre-cast to bf16 once and stay device-resident across calls.
Passthrough rows never leave the host.
"""
import numpy as np

import concourse.bass as bass
import concourse.mybir as mybir
import concourse.tile as tile
from concourse import bacc
from concourse.bass_utils import run_bass_kernel_spmd

P = 128
B, S, D, H = 2, 4096, 1024, 16
HD = D // H           # 64
DFF = 4 * D           # 4096
M = 511               # selected tokens
MT = 512              # padded
DG = D // P           # 8 feature groups
NF = DFF // 1024      # MLP chunks
PKR = B * MT + B * 2 * 16   # packed input rows: tokens + cos/sin (16 rows each)
NEG = -1e9
EPS = 1e-5

FP = mybir.dt.float32
BF = mybir.dt.bfloat16

AL = mybir.AluOpType
AF = mybir.ActivationFunctionType

_NC_CACHE = {}
_RUN_CACHE = {}


def _build_nc():
    if "nc" in _NC_CACHE:
        return _NC_CACHE["nc"]
    nc = bacc.Bacc("TRN2", target_bir_lowering=False, debug=False)

    T = {}

    def din(name, shape, dt):
        T[name] = nc.dram_tensor(name, shape, dt, kind="ExternalInput")

    def dout(name, shape, dt):
        T[name] = nc.dram_tensor(name, shape, dt, kind="ExternalOutput")

    # per-call packed buffer: rows [0,1024) tokens (b0|b1); then per batch
    # 16 rows cos + 16 rows sin ([32,512] reshaped to [16,1024]).
    din("pk", [PKR, D], BF)
    # resident (uploaded once, cached on device across calls)
    din("wqd", [D, D], BF)
    din("wkd", [D, D], BF)
    din("wvd", [D, D], BF)
    din("wod", [D, D], BF)
    din("w1d", [D, DFF], BF)
    din("w2d", [DFF, D], BF)
    din("ln1g", [P, D], FP)
    din("ln1b", [P, D], FP)
    din("ln2g", [P, D], FP)
    din("ln2b", [P, D], FP)
    din("onr_d", [1, P], FP)
    din("idb_d", [P, P], BF)
    din("tri_d", [P, MT], FP)

    dout("x3o", [B * MT, D], BF)

    with tile.TileContext(nc) as tc:
        _emit(nc, tc, T)
    nc.compile()
    _NC_CACHE["nc"] = nc
    return nc


def _emit(nc, tc, T):
    import contextlib
    with contextlib.ExitStack() as ctx:
        const = ctx.enter_context(tc.tile_pool(name="const", bufs=1))
        sb = ctx.enter_context(tc.tile_pool(name="sb", bufs=1))
        sb2 = ctx.enter_context(tc.tile_pool(name="sb2", bufs=2))
        stage = ctx.enter_context(tc.tile_pool(name="stage", bufs=3))
        wts = ctx.enter_context(tc.tile_pool(name="wts", bufs=2))
        # PSUM: mm(3) + mmb(1) + sc(2) + ctx(2) = 8 banks
        ppmm = ctx.enter_context(tc.tile_pool(name="ppmm", bufs=3, space="PSUM"))
        ppmb = ctx.enter_context(tc.tile_pool(name="ppmb", bufs=1, space="PSUM"))
        ppsc = ctx.enter_context(tc.tile_pool(name="ppsc", bufs=1, space="PSUM"))
        ppcx = ctx.enter_context(tc.tile_pool(name="ppcx", bufs=2, space="PSUM"))

        def cload(name, shape, dt):
            t = const.tile(shape, dt, tag=name, name=f"c_{name}")
            nc.sync.dma_start(t[:], T[name][:])
            return t

        C = {
            "onr": cload("onr_d", [1, P], FP),
            "idb": cload("idb_d", [P, P], BF),
            "tri": cload("tri_d", [P, MT], FP),
            "l1g": cload("ln1g", [P, D], FP),
            "l1b": cload("ln1b", [P, D], FP),
            "l2g": cload("ln2g", [P, D], FP),
            "l2b": cload("ln2b", [P, D], FP),
        }
        pools = (sb, sb2, stage, wts, ppmm, ppmb, ppsc, ppcx)
        for b in range(B):
            _emit_block(nc, T, C, pools, b)


def _emit_block(nc, T, C, pools, b):
    sb, sb2, stage, wts, ppmm, ppmb, ppsc, ppcx = pools
    onr, idb, tri = C["onr"], C["idb"], C["tri"]

    # ---------- RoPE tables: [16,1024] packed rows -> [32,512] -> x4 ----------
    cr0 = B * MT + b * 32
    cosC = sb.tile([32, MT], BF, tag="cosC")
    nc.sync.dma_start(cosC[:], T["pk"][cr0:cr0 + 16, :]
                      .rearrange("r (s c) -> (r s) c", s=2))
    sinC = sb.tile([32, MT], BF, tag="sinC")
    nc.sync.dma_start(sinC[:], T["pk"][cr0 + 16:cr0 + 32, :]
                      .rearrange("r (s c) -> (r s) c", s=2))
    cosT = sb.tile([P, MT], BF, tag="cosT")
    sinT = sb.tile([P, MT], BF, tag="sinT")
    for bb in range(4):
        nc.scalar.copy(cosT[32 * bb:32 * (bb + 1), :], cosC[:])
        nc.scalar.copy(sinT[32 * bb:32 * (bb + 1), :], sinC[:])

    # ---------- selected tokens (natural layout, t = g*128 + p) ----------
    tokb = sb.tile([P, 4, D], BF, tag="tokb")
    nc.sync.dma_start(tokb[:], T["pk"][b * MT:(b + 1) * MT, :]
                      .rearrange("(g p) d -> p g d", p=P))
    x1 = sb.tile([P, 4, D], FP, tag="x1")
    nc.vector.tensor_copy(x1[:], tokb[:])

    # ---------- LN1 ----------
    h_bf = sb.tile([P, 4, D], BF, tag="actN")
    _layernorm(nc, sb, stage, x1, h_bf, C["l1g"], C["l1b"])

    # ---------- transpose h ----------
    hT = sb.tile([P, DG, MT], BF, tag="actT")
    _transpose_nat_to_T(nc, ppmb, h_bf, hT, idb)

    def wload(dram, col0, cols):
        wt = wts.tile([P, DG, cols], BF, tag="w")
        for dg in range(DG):
            nc.sync.dma_start(wt[:, dg, :],
                              dram[dg * P:(dg + 1) * P, col0:col0 + cols])
        return wt

    # ---------- QKV (transposed) + RoPE in place ----------
    # Wq is pre-scaled by 1/sqrt(HD) on the host, so q/k RoPE share cos/sin.
    wq_bf = wload(T["wqd"], 0, D)
    qT = sb.tile([P, DG, MT], BF, tag="qT")
    _proj_T(nc, ppmm, wq_bf, hT, qT)
    wk_bf = wload(T["wkd"], 0, D)
    kT = sb.tile([P, DG, MT], BF, tag="kT")
    _proj_T(nc, ppmm, wk_bf, hT, kT)
    _rope(nc, sb, qT, cosT, sinT)
    _rope(nc, sb, kT, cosT, sinT)

    # ---------- V natural + interleaved ones ----------
    wv_bf = wload(T["wvd"], 0, D)
    vN2 = sb.tile([P, 4, H * (HD + 1)], BF, tag="vN2")
    for tc_ in range(4):
        for half in range(2):
            vp = ppmm.tile([P, MT], FP, tag="mm")
            for dg in range(DG):
                nc.tensor.matmul(
                    out=vp[:], lhsT=hT[:, dg, tc_ * P:(tc_ + 1) * P],
                    rhs=wv_bf[:, dg, half * 512:(half + 1) * 512],
                    start=(dg == 0), stop=(dg == DG - 1))
            dst = vN2[:, tc_, :].rearrange("p (h e) -> p h e", e=HD + 1)
            nc.scalar.copy(dst[:, half * 8:(half + 1) * 8, 0:HD],
                           vp[:].rearrange("p (h e) -> p h e", e=HD))
    nc.vector.memset(
        vN2[:, :, :].rearrange("p g (h e) -> p g h e", e=HD + 1)[:, :, :, HD:HD + 1],
        1.0)

    # ---------- attention (waves of 2 heads) ----------
    ctxT = sb.tile([P, DG, MT], BF, tag="ctxT")
    for wv_ in range(8):
        scps = ppsc.tile([P, 2, MT], FP, tag="sc")
        expb = sb2.tile([P, 2, MT], BF, tag="expb")
        ctps = [ppcx.tile([HD + 1, MT], FP, tag="cx", name=f"ctps{b}_{wv_}_{j}")
                for j in range(2)]
        for kt in range(4):
            qt0 = P * kt
            qtw = MT - qt0
            for j in range(2):
                h = 2 * wv_ + j
                m, o = h // 2, HD * (h % 2)
                nc.tensor.matmul(
                    out=scps[:, j, qt0:MT],
                    lhsT=kT[o:o + HD, m, kt * P:(kt + 1) * P],
                    rhs=qT[o:o + HD, m, qt0:MT],
                    start=True, stop=True)
            nc.vector.tensor_tensor(
                out=scps[:, :, qt0:MT], in0=scps[:, :, qt0:MT],
                in1=tri[:, None, 0:qtw].to_broadcast([P, 2, qtw]),
                op=AL.add)
            nc.scalar.activation(expb[:, :, qt0:MT], scps[:, :, qt0:MT], AF.Exp)
            for j in range(2):
                h = 2 * wv_ + j
                nc.tensor.matmul(
                    out=ctps[j][:, qt0:MT],
                    lhsT=vN2[:, kt, h * (HD + 1):(h + 1) * (HD + 1)],
                    rhs=expb[:, j, qt0:MT],
                    start=(kt == 0), stop=(kt == 3))
        for j in range(2):
            h = 2 * wv_ + j
            m, o = h // 2, HD * (h % 2)
            rec = sb2.tile([1, MT], FP, tag="rec")
            nc.vector.reciprocal(rec[:], ctps[j][HD:HD + 1, :])
            rbps = ppmb.tile([HD, MT], FP, tag="mmb")
            nc.tensor.matmul(out=rbps[:], lhsT=onr[0:1, 0:HD], rhs=rec[:],
                             start=True, stop=True)
            rbsb = sb2.tile([HD, MT], FP, tag="rbsb")
            nc.scalar.copy(rbsb[:], rbps[:])
            nc.vector.tensor_tensor(out=ctxT[o:o + HD, m, :],
                                    in0=ctps[j][0:HD, :], in1=rbsb[:],
                                    op=AL.mult)

    # ---------- Wo + residual (x2 accumulated in place into x1) ----------
    wo_bf = wload(T["wod"], 0, D)
    for tc_ in range(4):
        for half in range(2):
            wops = ppmm.tile([P, MT], FP, tag="mm")
            for hg in range(DG):
                nc.tensor.matmul(
                    out=wops[:], lhsT=ctxT[:, hg, tc_ * P:(tc_ + 1) * P],
                    rhs=wo_bf[:, hg, half * 512:(half + 1) * 512],
                    start=(hg == 0), stop=(hg == DG - 1))
            nc.vector.tensor_add(
                out=x1[:, tc_, half * 512:(half + 1) * 512],
                in0=x1[:, tc_, half * 512:(half + 1) * 512], in1=wops[:])

    # ---------- LN2 + transpose ----------
    h2_bf = sb.tile([P, 4, D], BF, tag="actN2")
    _layernorm(nc, sb, stage, x1, h2_bf, C["l2g"], C["l2b"])
    h2T = sb.tile([P, DG, MT], BF, tag="actT2")
    _transpose_nat_to_T(nc, ppmb, h2_bf, h2T, idb)

    # ---------- full-DFF MLP, streamed in NF chunks of 1024 ----------
    for c in range(NF):
        w1c = wload(T["w1d"], c * 1024, 1024)
        geluT = sb2.tile([P, DG, MT], BF, tag="gel")
        for fm in range(DG):
            h1ps = ppmm.tile([P, MT], FP, tag="mm")
            for dg in range(DG):
                nc.tensor.matmul(
                    out=h1ps[:], lhsT=w1c[:, dg, fm * P:(fm + 1) * P],
                    rhs=h2T[:, dg, :],
                    start=(dg == 0), stop=(dg == DG - 1))
            nc.scalar.activation(geluT[:, fm, :], h1ps[:], AF.Gelu_apprx_tanh)
        w2c = wts.tile([P, DG, D], BF, tag="w")
        for dg in range(DG):
            nc.sync.dma_start(
                w2c[:, dg, :],
                T["w2d"][c * 1024 + dg * P:c * 1024 + (dg + 1) * P, :])
        for tc_ in range(4):
            for half in range(2):
                m2ps = ppmm.tile([P, MT], FP, tag="mm")
                for fg in range(DG):
                    nc.tensor.matmul(
                        out=m2ps[:], lhsT=geluT[:, fg, tc_ * P:(tc_ + 1) * P],
                        rhs=w2c[:, fg, half * 512:(half + 1) * 512],
                        start=(fg == 0), stop=(fg == DG - 1))
                nc.vector.tensor_add(
                    out=x1[:, tc_, half * 512:(half + 1) * 512],
                    in0=x1[:, tc_, half * 512:(half + 1) * 512],
                    in1=m2ps[:])

    x3bf = sb.tile([P, 4, D], BF, tag="x3bf")
    nc.scalar.copy(x3bf[:], x1[:])
    nc.sync.dma_start(
        T["x3o"][b * MT:(b + 1) * MT, :].rearrange("(g p) d -> p g d", p=P),
        x3bf[:])


def _layernorm(nc, sb, stage, x, out_bf, g_rep, b_rep):
    """x [128, 4, D] f32 -> out_bf [128, 4, D] bf16 = LN(x)*g + b."""
    stat = sb.tile([P, 4], FP, tag="lnsum")
    nc.vector.tensor_reduce(out=stat[:], in_=x[:], axis=mybir.AxisListType.X,
                            op=AL.add)
    mu = sb.tile([P, 4], FP, tag="lnmu")
    nc.vector.tensor_scalar_mul(mu[:], stat[:], 1.0 / D)
    var = sb.tile([P, 4], FP, tag="lnvar")
    for g in range(4):
        xc = stage.tile([P, D], FP, tag="stg")
        nc.vector.tensor_scalar(out=xc[:], in0=x[:, g, :],
                                scalar1=mu[:, g:g + 1], scalar2=None,
                                op0=AL.subtract)
        jt = stage.tile([P, D], FP, tag="stg")
        nc.vector.tensor_mul(jt[:], xc[:], xc[:])
        nc.vector.tensor_reduce(out=var[:, g:g + 1], in_=jt[:],
                                axis=mybir.AxisListType.X, op=AL.add)
    sd = sb.tile([P, 4], FP, tag="lnsd")
    nc.vector.tensor_scalar(out=sd[:], in0=var[:], scalar1=1.0 / D, scalar2=EPS,
                            op0=AL.mult, op1=AL.add)
    nc.scalar.sqrt(sd[:], sd[:])
    rstd = sb.tile([P, 4], FP, tag="lnrstd")
    nc.vector.reciprocal(rstd[:], sd[:])
    for g in range(4):
        xc = stage.tile([P, D], FP, tag="stg")
        nc.vector.tensor_scalar(out=xc[:], in0=x[:, g, :],
                                scalar1=mu[:, g:g + 1], scalar2=None,
                                op0=AL.subtract)
        nc.vector.tensor_scalar(out=xc[:], in0=xc[:],
                                scalar1=rstd[:, g:g + 1], scalar2=None,
                                op0=AL.mult)
        nc.vector.tensor_mul(out=xc[:], in0=xc[:], in1=g_rep[:])
        nc.vector.tensor_tensor(out=out_bf[:, g, :], in0=xc[:],
                                in1=b_rep[:], op=AL.add)


def _transpose_nat_to_T(nc, ppmb, nat_bf, outT, idb):
    """[128(tok), 4, D] bf16 -> [128(d), 8, 512(tok)] bf16 via PE."""
    for g in range(4):
        for m in range(DG):
            tp = ppmb.tile([P, P], BF, tag="mmb")
            nc.tensor.transpose(out=tp[:], in_=nat_bf[:, g, m * P:(m + 1) * P],
                                identity=idb[:])
            nc.scalar.copy(outT[:, m, g * P:(g + 1) * P], tp[:])


def _proj_T(nc, ppmm, w_bf, hT, outT):
    """outT[128, 8, 512] = (h @ W)^T; W loaded [128, 8, D]."""
    for m in range(DG):
        pp = ppmm.tile([P, MT], FP, tag="mm")
        for dg in range(DG):
            nc.tensor.matmul(out=pp[:], lhsT=w_bf[:, dg, m * P:(m + 1) * P],
                             rhs=hT[:, dg, :],
                             start=(dg == 0), stop=(dg == DG - 1))
        nc.scalar.copy(outT[:, m, :], pp[:])


def _rope(nc, sbp, xT, cosv, sinv):
    """In-place RoPE on transposed q/k [128, 8, 512]; pairs (p, p+32)/64-block.

    Two half-passes over the middle dim to bound temp size.
    """
    for half in range(2):
        gs = slice(half * 4, half * 4 + 4)
        for base in (0, 64):
            cb = cosv[base:base + 32, None, :].to_broadcast([32, 4, MT])
            sbr = sinv[base:base + 32, None, :].to_broadcast([32, 4, MT])
            cb2 = cosv[base + 32:base + 64, None, :].to_broadcast([32, 4, MT])
            sb2r = sinv[base + 32:base + 64, None, :].to_broadcast([32, 4, MT])
            a1 = xT[base:base + 32, gs, :]
            a2 = xT[base + 32:base + 64, gs, :]
            t1c = sbp.tile([32, 4, MT], BF, tag="rp1")
            t1s = sbp.tile([32, 4, MT], BF, tag="rp2")
            t2s = sbp.tile([32, 4, MT], BF, tag="rp3")
            nc.vector.tensor_tensor(out=t1c[:], in0=a1, in1=cb, op=AL.mult)
            nc.vector.tensor_tensor(out=t1s[:], in0=a1, in1=sbr, op=AL.mult)
            nc.vector.tensor_tensor(out=t2s[:], in0=a2, in1=sb2r, op=AL.mult)
            # a1 <- a1*cos - a2*sin
            nc.vector.tensor_tensor(out=a1, in0=t1c[:], in1=t2s[:],
                                    op=AL.subtract)
            # a2 <- a1_old*sin + a2*cos
            nc.vector.tensor_tensor(out=t1c[:], in0=a2, in1=cb2, op=AL.mult)
            nc.vector.tensor_tensor(out=a2, in0=t1s[:], in1=t1c[:], op=AL.add)


# ======================= host side =======================

def _get_runner(nc):
    """Persistent jit wrapper over the bass_exec custom call (the same
    lowering run_bass_kernel_spmd uses under axon), kept across calls so the
    executable and device-resident params are reused instead of re-created."""
    if "runner" in _RUN_CACHE:
        return _RUN_CACHE["runner"]
    import jax
    import jax.numpy as jnp
    import concourse.bass2jax as b2j

    b2j.install_neuronx_cc_hook()
    partition_name = nc.partition_id_tensor.name if nc.partition_id_tensor else None
    in_names, out_names, out_avals, zero_shapes = [], [], [], []
    for alloc in nc.m.functions[0].allocations:
        if not isinstance(alloc, mybir.MemoryLocationSet):
            continue
        name = alloc.memorylocations[0].name
        if alloc.kind == "ExternalInput":
            if name != partition_name:
                in_names.append(name)
        elif alloc.kind == "ExternalOutput":
            out_names.append(name)
            shape = tuple(alloc.tensor_shape)
            dtype = mybir.dt.np(alloc.dtype)
            out_avals.append(jax.core.ShapedArray(shape, dtype))
            zero_shapes.append((shape, dtype))
    n_params = len(in_names)
    n_outs = len(out_avals)
    all_names = list(in_names) + list(out_names)
    if partition_name is not None:
        all_names.append(partition_name)
    donate = tuple(range(n_params, n_params + n_outs))

    def _body(*args):
        operands = list(args)
        if partition_name is not None:
            operands.append(b2j.partition_id_tensor())
        outs = b2j._bass_exec_p.bind(
            *operands,
            out_avals=tuple(out_avals),
            in_names=tuple(all_names),
            out_names=tuple(out_names),
            lowering_input_output_aliases=(),
            sim_require_finite=True,
            sim_require_nnan=True,
            nc=nc,
        )
        return tuple(outs)

    dev = jax.devices()[0]
    sds = jax.sharding.SingleDeviceSharding(dev)
    jf = jax.jit(_body, donate_argnums=donate, keep_unused=True)
    mkzeros = jax.jit(
        lambda: tuple(jnp.zeros(s, d) for s, d in zero_shapes),
        out_shardings=(sds,) * n_outs)
    runner = {
        "jf": jf, "in_names": in_names, "out_names": out_names,
        "out_avals": out_avals, "zero_shapes": zero_shapes,
        "dev": dev, "mkzeros": mkzeros,
        "resident": {},       # name -> device Array (shared params)
    }
    _RUN_CACHE["runner"] = runner
    return runner


def _run_fast(nc, shared, percall):
    """Execute with device-resident shared params; returns the output Arrays
    (call _fetch on the result to get the np dict)."""
    import jax
    r = _get_runner(nc)
    params = []
    for name in r["in_names"]:
        if name in shared:
            arr = r["resident"].get(name)
            if arr is None:
                arr = jax.device_put(np.asarray(shared[name]), r["dev"])
                r["resident"][name] = arr
            params.append(arr)
        else:
            params.append(np.asarray(percall[name]))
    zeros = r["mkzeros"]()
    return r["jf"](*params, *zeros)


def _fetch(out_arrs):
    r = _RUN_CACHE["runner"]
    return {name: np.asarray(out_arrs[i])
            for i, name in enumerate(r["out_names"])}


def _consts():
    import ml_dtypes
    c = {}
    c["onr_d"] = np.ones((1, P), np.float32)
    c["idb_d"] = np.eye(P).astype(ml_dtypes.bfloat16)
    p_ = np.arange(P)[:, None]
    f_ = np.arange(MT)[None, :]
    c["tri_d"] = np.where(p_ <= f_, 0.0, NEG).astype(np.float32)
    return c


def _route_host(hidden_states, router_w):
    """Exact replica of the reference routing, on jax CPU."""
    import jax
    import jax.numpy as jnp
    cpu = jax.devices("cpu")[0]
    with jax.default_device(cpu):
        w = jnp.einsum('bsd,d->bs', jnp.asarray(hidden_states),
                       jnp.asarray(router_w)[:, 0])
        k = MT
        top_vals, top_idx = jax.lax.top_k(w, k)
        sel_idx = jnp.sort(top_idx[:, :M], axis=1)
        return np.asarray(w), np.asarray(sel_idx)


def _make_shared(Wq, Wk, Wv, Wo, W1, W2, ln1_g, ln1_b, ln2_g, ln2_b):
    import ml_dtypes
    bf = lambda a: np.ascontiguousarray(
        np.asarray(a, np.float32).astype(ml_dtypes.bfloat16))
    rep = lambda v: np.ascontiguousarray(
        np.broadcast_to(np.asarray(v, np.float32)[None, :], (P, D)))
    return {
        "wqd": bf(np.asarray(Wq, np.float32) * (1.0 / np.sqrt(HD))),
        "wkd": bf(Wk), "wvd": bf(Wv), "wod": bf(Wo),
        "w1d": bf(W1), "w2d": bf(W2),
        "ln1g": rep(ln1_g), "ln1b": rep(ln1_b),
        "ln2g": rep(ln2_g), "ln2b": rep(ln2_b),
        **_consts(),
    }


def kernel(hidden_states, attention_mask, position_ids, router_w,
           Wq, Wk, Wv, Wo, W1, W2, ln1_g, ln1_b, ln2_g, ln2_b):
    import ml_dtypes
    hidden_states = np.ascontiguousarray(np.asarray(hidden_states, np.float32))
    router_w = np.asarray(router_w, np.float32)

    w, sel = _route_host(hidden_states, router_w)          # [B,S], [B,M]
    rw = w[np.arange(B)[:, None], sel]                     # [B,M]

    pos = np.broadcast_to(np.asarray(position_ids, np.int64), (B, S))
    inv = (1.0 / (10000.0 ** (np.arange(0, HD, 2, dtype=np.float32) / HD)))

    nc = _build_nc()

    # Shared (weight/const) params are cached device-resident; invalidate if
    # the caller passed different weight values than the resident copy.
    orig = (Wq, Wk, Wv, Wo, W1, W2, ln1_g, ln1_b, ln2_g, ln2_b)
    prev_refs = _RUN_CACHE.get("raw_refs")
    same = prev_refs is not None and all(
        p is r for p, r in zip(prev_refs, orig))
    if not same:
        raw = [np.asarray(a, np.float32) for a in orig]
        prev = _RUN_CACHE.get("raw_weights")
        same = prev is not None and all(
            p.shape == r.shape and np.array_equal(p, r)
            for p, r in zip(prev, raw))
        if not same:
            _RUN_CACHE["raw_weights"] = [np.array(a, copy=True) for a in raw]
            _RUN_CACHE["shared"] = _make_shared(*raw)
            if "runner" in _RUN_CACHE:
                _RUN_CACHE["runner"]["resident"].clear()
        _RUN_CACHE["raw_refs"] = list(orig)
    shared = _RUN_CACHE["shared"]

    # packed per-call buffer: tokens (both batches) + cos/sin tables
    pk = np.zeros((PKR, D), ml_dtypes.bfloat16)
    for b in range(B):
        pk[b * MT:b * MT + M] = hidden_states[b, sel[b]].astype(ml_dtypes.bfloat16)
        sel_pos = np.zeros((MT,), np.float32)
        sel_pos[:M] = pos[b, sel[b]].astype(np.float32)
        ang = sel_pos[:, None] * inv[None, :]              # [MT, 32]
        cr0 = B * MT + b * 32
        pk[cr0:cr0 + 16] = (np.cos(ang).astype(np.float32).T
                            .astype(ml_dtypes.bfloat16).reshape(16, D))
        pk[cr0 + 16:cr0 + 32] = (np.sin(ang).astype(np.float32).T
                                 .astype(ml_dtypes.bfloat16).reshape(16, D))

    import os
    trace = os.environ.get("BASS_TRACE") and not os.environ.get("BASS_NEVER_TRACE")
    results = None
    if not trace:
        try:
            out_arrs = _run_fast(nc, shared, {"pk": pk})
            out = np.array(hidden_states, copy=True)   # overlaps device exec
            results = _fetch(out_arrs)
        except Exception:
            _RUN_CACHE.pop("runner", None)
            results = None
    if results is None:
        full_map = {**shared, "pk": pk}
        res = run_bass_kernel_spmd(nc, [full_map], core_ids=[0])
        results = {"x3o": res.results[0]["x3o"]}
        out = np.array(hidden_states, copy=True)

    x3 = results["x3o"]
    for b in range(B):
        xb = x3[b * MT:b * MT + M].astype(np.float32)
        out[b, sel[b]] = xb * rw[b][:, None]
    return out


# revision 27
# speedup vs baseline: 1.5198x; 1.5198x over previous
"""MixtureOfDepth Trainium2 Bass kernel (2-core SPMD, host-side routing).

Router matvec, top-k threshold selection, token gather and scatter are
per-batch-row independent and tiny, so they run on the host with the exact
same jax CPU ops as the reference (bit-identical selection). The device
(core b = batch b) runs only the dense transformer block on the 511
selected tokens: pre-LN attention with RoPE (bf16 matmuls, f32 accum) and
the full-DFF MLP with W1/W2 streamed in 1024-column chunks. Weights ship
pre-cast to bf16; passthrough rows never leave the host.
"""
import numpy as np

import concourse.bass as bass
import concourse.mybir as mybir
import concourse.tile as tile
from concourse import bacc
from concourse.bass_utils import run_bass_kernel_spmd

P = 128
B, S, D, H = 2, 4096, 1024, 16
HD = D // H           # 64
DFF = 4 * D           # 4096
M = 511               # selected tokens
MT = 512              # padded
DG = D // P           # 8 feature groups
NF = DFF // 1024      # MLP chunks
NEG = -1e9
EPS = 1e-5

FP = mybir.dt.float32
BF = mybir.dt.bfloat16

AL = mybir.AluOpType
AF = mybir.ActivationFunctionType

_NC_CACHE = {}


def _build_nc():
    if "nc" in _NC_CACHE:
        return _NC_CACHE["nc"]
    nc = bacc.Bacc("TRN2", target_bir_lowering=False, debug=False)

    T = {}

    def din(name, shape, dt):
        T[name] = nc.dram_tensor(name, shape, dt, kind="ExternalInput")

    def dout(name, shape, dt):
        T[name] = nc.dram_tensor(name, shape, dt, kind="ExternalOutput")

    din("tok", [MT, D], BF)
    din("cosT_d", [32, MT], FP)
    din("sinT_d", [32, MT], FP)
    din("wqd", [D, D], BF)
    din("wkd", [D, D], BF)
    din("wvd", [D, D], BF)
    din("wod", [D, D], BF)
    din("w1d", [D, DFF], BF)
    din("w2d", [DFF, D], BF)
    din("ln1g", [P, D], FP)
    din("ln1b", [P, D], FP)
    din("ln2g", [P, D], FP)
    din("ln2b", [P, D], FP)
    din("onr_d", [1, P], FP)
    din("idb_d", [P, P], BF)
    din("tri_d", [P, MT], FP)

    dout("x3o", [MT, D], BF)

    with tile.TileContext(nc) as tc:
        _emit(nc, tc, T)
    nc.compile()
    _NC_CACHE["nc"] = nc
    return nc


def _emit(nc, tc, T):
    import contextlib
    with contextlib.ExitStack() as ctx:
        const = ctx.enter_context(tc.tile_pool(name="const", bufs=1))
        sb = ctx.enter_context(tc.tile_pool(name="sb", bufs=1))
        sb2 = ctx.enter_context(tc.tile_pool(name="sb2", bufs=2))
        stage = ctx.enter_context(tc.tile_pool(name="stage", bufs=3))
        wts = ctx.enter_context(tc.tile_pool(name="wts", bufs=2))
        # PSUM: mm(3) + mmb(1) + sc(2) + ctx(2) = 8 banks
        ppmm = ctx.enter_context(tc.tile_pool(name="ppmm", bufs=3, space="PSUM"))
        ppmb = ctx.enter_context(tc.tile_pool(name="ppmb", bufs=1, space="PSUM"))
        ppsc = ctx.enter_context(tc.tile_pool(name="ppsc", bufs=1, space="PSUM"))
        ppcx = ctx.enter_context(tc.tile_pool(name="ppcx", bufs=2, space="PSUM"))

        def cload(name, shape, dt):
            t = const.tile(shape, dt, tag=name, name=f"c_{name}")
            nc.sync.dma_start(t[:], T[name][:])
            return t

        onr = cload("onr_d", [1, P], FP)
        idb = cload("idb_d", [P, P], BF)
        tri = cload("tri_d", [P, MT], FP)
        # RoPE tables ship compact [32, MT]; replicate to all four 32-row
        # blocks on device (TensorTensor needs matching input partitions).
        cosC = cload("cosT_d", [32, MT], FP)
        sinC = cload("sinT_d", [32, MT], FP)
        cosT = sb.tile([P, MT], FP, tag="cosT")
        sinT = sb.tile([P, MT], FP, tag="sinT")
        for bb in range(4):
            nc.scalar.copy(cosT[32 * bb:32 * (bb + 1), :], cosC[:])
            nc.scalar.copy(sinT[32 * bb:32 * (bb + 1), :], sinC[:])
        l1g = cload("ln1g", [P, D], FP)
        l1b = cload("ln1b", [P, D], FP)
        l2g = cload("ln2g", [P, D], FP)
        l2b = cload("ln2b", [P, D], FP)

        # ---------- selected tokens (natural layout, t = g*128 + p) ----------
        tokb = sb.tile([P, 4, D], BF, tag="tokb")
        nc.sync.dma_start(tokb[:], T["tok"][:].rearrange("(g p) d -> p g d", p=P))
        x1 = sb.tile([P, 4, D], FP, tag="x1")
        nc.vector.tensor_copy(x1[:], tokb[:])

        # ---------- LN1 ----------
        h_bf = sb.tile([P, 4, D], BF, tag="actN")
        _layernorm(nc, sb, stage, x1, h_bf, l1g, l1b)

        # ---------- transpose h ----------
        hT = sb.tile([P, DG, MT], BF, tag="actT")
        _transpose_nat_to_T(nc, ppmb, h_bf, hT, idb)

        def wload(dram, col0, cols):
            wt = wts.tile([P, DG, cols], BF, tag="w")
            for dg in range(DG):
                nc.sync.dma_start(wt[:, dg, :],
                                  dram[dg * P:(dg + 1) * P, col0:col0 + cols])
            return wt

        # ---------- QKV (transposed) + RoPE in place ----------
        # Wq is pre-scaled by 1/sqrt(HD) on the host, so q/k RoPE share cos/sin.
        wq_bf = wload(T["wqd"], 0, D)
        qT = sb.tile([P, DG, MT], BF, tag="qT")
        _proj_T(nc, ppmm, wq_bf, hT, qT)
        wk_bf = wload(T["wkd"], 0, D)
        kT = sb.tile([P, DG, MT], BF, tag="kT")
        _proj_T(nc, ppmm, wk_bf, hT, kT)
        _rope(nc, sb, qT, cosT, sinT)
        _rope(nc, sb, kT, cosT, sinT)

        # ---------- V natural + interleaved ones ----------
        wv_bf = wload(T["wvd"], 0, D)
        vN2 = sb.tile([P, 4, H * (HD + 1)], BF, tag="vN2")
        for tc_ in range(4):
            for half in range(2):
                vp = ppmm.tile([P, MT], FP, tag="mm")
                for dg in range(DG):
                    nc.tensor.matmul(
                        out=vp[:], lhsT=hT[:, dg, tc_ * P:(tc_ + 1) * P],
                        rhs=wv_bf[:, dg, half * 512:(half + 1) * 512],
                        start=(dg == 0), stop=(dg == DG - 1))
                dst = vN2[:, tc_, :].rearrange("p (h e) -> p h e", e=HD + 1)
                nc.scalar.copy(dst[:, half * 8:(half + 1) * 8, 0:HD],
                               vp[:].rearrange("p (h e) -> p h e", e=HD))
        nc.vector.memset(
            vN2[:, :, :].rearrange("p g (h e) -> p g h e", e=HD + 1)[:, :, :, HD:HD + 1],
            1.0)

        # ---------- attention (waves of 2 heads) ----------
        ctxT = sb.tile([P, DG, MT], BF, tag="ctxT")
        for wv_ in range(8):
            scps = ppsc.tile([P, 2, MT], FP, tag="sc")
            expb = sb2.tile([P, 2, MT], BF, tag="expb")
            ctps = [ppcx.tile([HD + 1, MT], FP, tag="cx", name=f"ctps{wv_}_{j}")
                    for j in range(2)]
            for kt in range(4):
                qt0 = P * kt
                qtw = MT - qt0
                for j in range(2):
                    h = 2 * wv_ + j
                    m, o = h // 2, HD * (h % 2)
                    nc.tensor.matmul(
                        out=scps[:, j, qt0:MT],
                        lhsT=kT[o:o + HD, m, kt * P:(kt + 1) * P],
                        rhs=qT[o:o + HD, m, qt0:MT],
                        start=True, stop=True)
                nc.vector.tensor_tensor(
                    out=scps[:, :, qt0:MT], in0=scps[:, :, qt0:MT],
                    in1=tri[:, None, 0:qtw].to_broadcast([P, 2, qtw]),
                    op=AL.add)
                nc.scalar.activation(expb[:, :, qt0:MT], scps[:, :, qt0:MT], AF.Exp)
                for j in range(2):
                    h = 2 * wv_ + j
                    nc.tensor.matmul(
                        out=ctps[j][:, qt0:MT],
                        lhsT=vN2[:, kt, h * (HD + 1):(h + 1) * (HD + 1)],
                        rhs=expb[:, j, qt0:MT],
                        start=(kt == 0), stop=(kt == 3))
            for j in range(2):
                h = 2 * wv_ + j
                m, o = h // 2, HD * (h % 2)
                rec = sb2.tile([1, MT], FP, tag="rec")
                nc.vector.reciprocal(rec[:], ctps[j][HD:HD + 1, :])
                rbps = ppmb.tile([HD, MT], FP, tag="mmb")
                nc.tensor.matmul(out=rbps[:], lhsT=onr[0:1, 0:HD], rhs=rec[:],
                                 start=True, stop=True)
                rbsb = sb2.tile([HD, MT], FP, tag="rbsb")
                nc.scalar.copy(rbsb[:], rbps[:])
                nc.vector.tensor_tensor(out=ctxT[o:o + HD, m, :],
                                        in0=ctps[j][0:HD, :], in1=rbsb[:],
                                        op=AL.mult)

        # ---------- Wo + residual (x2 accumulated in place into x1) ----------
        wo_bf = wload(T["wod"], 0, D)
        for tc_ in range(4):
            for half in range(2):
                wops = ppmm.tile([P, MT], FP, tag="mm")
                for hg in range(DG):
                    nc.tensor.matmul(
                        out=wops[:], lhsT=ctxT[:, hg, tc_ * P:(tc_ + 1) * P],
                        rhs=wo_bf[:, hg, half * 512:(half + 1) * 512],
                        start=(hg == 0), stop=(hg == DG - 1))
                nc.vector.tensor_add(
                    out=x1[:, tc_, half * 512:(half + 1) * 512],
                    in0=x1[:, tc_, half * 512:(half + 1) * 512], in1=wops[:])

        # ---------- LN2 + transpose ----------
        h2_bf = sb.tile([P, 4, D], BF, tag="actN2")
        _layernorm(nc, sb, stage, x1, h2_bf, l2g, l2b)
        h2T = sb.tile([P, DG, MT], BF, tag="actT2")
        _transpose_nat_to_T(nc, ppmb, h2_bf, h2T, idb)

        # ---------- full-DFF MLP, streamed in NF chunks of 1024 ----------
        for c in range(NF):
            w1c = wload(T["w1d"], c * 1024, 1024)
            geluT = sb2.tile([P, DG, MT], BF, tag="gel")
            for fm in range(DG):
                h1ps = ppmm.tile([P, MT], FP, tag="mm")
                for dg in range(DG):
                    nc.tensor.matmul(
                        out=h1ps[:], lhsT=w1c[:, dg, fm * P:(fm + 1) * P],
                        rhs=h2T[:, dg, :],
                        start=(dg == 0), stop=(dg == DG - 1))
                nc.scalar.activation(geluT[:, fm, :], h1ps[:], AF.Gelu_apprx_tanh)
            w2c = wts.tile([P, DG, D], BF, tag="w")
            for dg in range(DG):
                nc.sync.dma_start(
                    w2c[:, dg, :],
                    T["w2d"][c * 1024 + dg * P:c * 1024 + (dg + 1) * P, :])
            for tc_ in range(4):
                for half in range(2):
                    m2ps = ppmm.tile([P, MT], FP, tag="mm")
                    for fg in range(DG):
                        nc.tensor.matmul(
                            out=m2ps[:], lhsT=geluT[:, fg, tc_ * P:(tc_ + 1) * P],
                            rhs=w2c[:, fg, half * 512:(half + 1) * 512],
                            start=(fg == 0), stop=(fg == DG - 1))
                    nc.vector.tensor_add(
                        out=x1[:, tc_, half * 512:(half + 1) * 512],
                        in0=x1[:, tc_, half * 512:(half + 1) * 512],
                        in1=m2ps[:])

        x3bf = sb.tile([P, 4, D], BF, tag="x3bf")
        nc.scalar.copy(x3bf[:], x1[:])
        nc.sync.dma_start(T["x3o"][:].rearrange("(g p) d -> p g d", p=P), x3bf[:])


def _layernorm(nc, sb, stage, x, out_bf, g_rep, b_rep):
    """x [128, 4, D] f32 -> out_bf [128, 4, D] bf16 = LN(x)*g + b."""
    stat = sb.tile([P, 4], FP, tag="lnsum")
    nc.vector.tensor_reduce(out=stat[:], in_=x[:], axis=mybir.AxisListType.X,
                            op=AL.add)
    mu = sb.tile([P, 4], FP, tag="lnmu")
    nc.vector.tensor_scalar_mul(mu[:], stat[:], 1.0 / D)
    var = sb.tile([P, 4], FP, tag="lnvar")
    for g in range(4):
        xc = stage.tile([P, D], FP, tag="stg")
        nc.vector.tensor_scalar(out=xc[:], in0=x[:, g, :],
                                scalar1=mu[:, g:g + 1], scalar2=None,
                                op0=AL.subtract)
        jt = stage.tile([P, D], FP, tag="stg")
        nc.vector.tensor_mul(jt[:], xc[:], xc[:])
        nc.vector.tensor_reduce(out=var[:, g:g + 1], in_=jt[:],
                                axis=mybir.AxisListType.X, op=AL.add)
    sd = sb.tile([P, 4], FP, tag="lnsd")
    nc.vector.tensor_scalar(out=sd[:], in0=var[:], scalar1=1.0 / D, scalar2=EPS,
                            op0=AL.mult, op1=AL.add)
    nc.scalar.sqrt(sd[:], sd[:])
    rstd = sb.tile([P, 4], FP, tag="lnrstd")
    nc.vector.reciprocal(rstd[:], sd[:])
    for g in range(4):
        xc = stage.tile([P, D], FP, tag="stg")
        nc.vector.tensor_scalar(out=xc[:], in0=x[:, g, :],
                                scalar1=mu[:, g:g + 1], scalar2=None,
                                op0=AL.subtract)
        nc.vector.tensor_scalar(out=xc[:], in0=xc[:],
                                scalar1=rstd[:, g:g + 1], scalar2=None,
                                op0=AL.mult)
        nc.vector.tensor_mul(out=xc[:], in0=xc[:], in1=g_rep[:])
        nc.vector.tensor_tensor(out=out_bf[:, g, :], in0=xc[:],
                                in1=b_rep[:], op=AL.add)


def _transpose_nat_to_T(nc, ppmb, nat_bf, outT, idb):
    """[128(tok), 4, D] bf16 -> [128(d), 8, 512(tok)] bf16 via PE."""
    for g in range(4):
        for m in range(DG):
            tp = ppmb.tile([P, P], BF, tag="mmb")
            nc.tensor.transpose(out=tp[:], in_=nat_bf[:, g, m * P:(m + 1) * P],
                                identity=idb[:])
            nc.scalar.copy(outT[:, m, g * P:(g + 1) * P], tp[:])


def _proj_T(nc, ppmm, w_bf, hT, outT):
    """outT[128, 8, 512] = (h @ W)^T; W loaded [128, 8, D]."""
    for m in range(DG):
        pp = ppmm.tile([P, MT], FP, tag="mm")
        for dg in range(DG):
            nc.tensor.matmul(out=pp[:], lhsT=w_bf[:, dg, m * P:(m + 1) * P],
                             rhs=hT[:, dg, :],
                             start=(dg == 0), stop=(dg == DG - 1))
        nc.scalar.copy(outT[:, m, :], pp[:])


def _rope(nc, sbp, xT, cosv, sinv):
    """In-place RoPE on transposed q/k [128, 8, 512]; pairs (p, p+32)/64-block.

    Two half-passes over the middle dim to bound temp size.
    """
    for half in range(2):
        gs = slice(half * 4, half * 4 + 4)
        for base in (0, 64):
            cb = cosv[base:base + 32, None, :].to_broadcast([32, 4, MT])
            sbr = sinv[base:base + 32, None, :].to_broadcast([32, 4, MT])
            cb2 = cosv[base + 32:base + 64, None, :].to_broadcast([32, 4, MT])
            sb2r = sinv[base + 32:base + 64, None, :].to_broadcast([32, 4, MT])
            a1 = xT[base:base + 32, gs, :]
            a2 = xT[base + 32:base + 64, gs, :]
            t1c = sbp.tile([32, 4, MT], BF, tag="rp1")
            t1s = sbp.tile([32, 4, MT], BF, tag="rp2")
            t2s = sbp.tile([32, 4, MT], BF, tag="rp3")
            nc.vector.tensor_tensor(out=t1c[:], in0=a1, in1=cb, op=AL.mult)
            nc.vector.tensor_tensor(out=t1s[:], in0=a1, in1=sbr, op=AL.mult)
            nc.vector.tensor_tensor(out=t2s[:], in0=a2, in1=sb2r, op=AL.mult)
            # a1 <- a1*cos - a2*sin
            nc.vector.tensor_tensor(out=a1, in0=t1c[:], in1=t2s[:],
                                    op=AL.subtract)
            # a2 <- a1_old*sin + a2*cos
            nc.vector.tensor_tensor(out=t1c[:], in0=a2, in1=cb2, op=AL.mult)
            nc.vector.tensor_tensor(out=a2, in0=t1s[:], in1=t1c[:], op=AL.add)


# ======================= host side =======================

_RUN_CACHE = {}


def _get_runner(nc):
    """Persistent jit wrapper over the bass_exec custom call (the same
    lowering run_bass_kernel_spmd uses under axon), kept across calls so the
    executable and device-resident params are reused instead of re-created."""
    if "runner" in _RUN_CACHE:
        return _RUN_CACHE["runner"]
    import jax
    from jax.sharding import Mesh, PartitionSpec, NamedSharding
    from jax.experimental.shard_map import shard_map
    import concourse.bass2jax as b2j

    b2j.install_neuronx_cc_hook()
    n_cores = 2
    partition_name = nc.partition_id_tensor.name if nc.partition_id_tensor else None
    in_names, out_names, out_avals, zero_shapes = [], [], [], []
    for alloc in nc.m.functions[0].allocations:
        if not isinstance(alloc, mybir.MemoryLocationSet):
            continue
        name = alloc.memorylocations[0].name
        if alloc.kind == "ExternalInput":
            if name != partition_name:
                in_names.append(name)
        elif alloc.kind == "ExternalOutput":
            out_names.append(name)
            shape = tuple(alloc.tensor_shape)
            dtype = mybir.dt.np(alloc.dtype)
            out_avals.append(jax.core.ShapedArray(shape, dtype))
            zero_shapes.append((shape, dtype))
    n_params = len(in_names)
    n_outs = len(out_avals)
    all_names = list(in_names) + list(out_names)
    if partition_name is not None:
        all_names.append(partition_name)
    donate = tuple(range(n_params, n_params + n_outs))

    def _body(*args):
        operands = list(args)
        if partition_name is not None:
            operands.append(b2j.partition_id_tensor())
        outs = b2j._bass_exec_p.bind(
            *operands,
            out_avals=tuple(out_avals),
            in_names=tuple(all_names),
            out_names=tuple(out_names),
            lowering_input_output_aliases=(),
            sim_require_finite=True,
            sim_require_nnan=True,
            nc=nc,
        )
        return tuple(outs)

    devices = jax.devices()[:n_cores]
    mesh = Mesh(np.asarray(devices), ("core",))
    spec = NamedSharding(mesh, PartitionSpec("core"))
    jf = jax.jit(
        shard_map(_body, mesh=mesh,
                  in_specs=(PartitionSpec("core"),) * (n_params + n_outs),
                  out_specs=(PartitionSpec("core"),) * n_outs,
                  check_rep=False),
        donate_argnums=donate, keep_unused=True,
    )
    import jax.numpy as jnp
    mkzeros = jax.jit(
        lambda: tuple(jnp.zeros((n_cores * s[0], *s[1:]), d)
                      for s, d in zero_shapes),
        out_shardings=(spec,) * n_outs,
    )
    runner = {
        "jf": jf, "in_names": in_names, "out_names": out_names,
        "out_avals": out_avals, "zero_shapes": zero_shapes,
        "n_cores": n_cores, "spec": spec, "mkzeros": mkzeros,
        "resident": {},       # name -> device Array (shared params)
    }
    _RUN_CACHE["runner"] = runner
    return runner


def _run_fast(nc, shared, in_maps):
    """Execute with device-resident shared params; returns the output Arrays
    (call _fetch on the result to get per-core np dicts)."""
    import jax
    r = _get_runner(nc)
    n_cores = r["n_cores"]
    params = []
    for name in r["in_names"]:
        if name in shared:
            arr = r["resident"].get(name)
            if arr is None:
                v = np.asarray(shared[name])
                stacked = np.concatenate([v] * n_cores, axis=0)
                arr = jax.device_put(stacked, r["spec"])
                r["resident"][name] = arr
            params.append(arr)
        else:
            params.append(np.concatenate(
                [np.asarray(m[name]) for m in in_maps], axis=0))
    zeros = r["mkzeros"]()
    return r["jf"](*params, *zeros)


def _fetch(out_arrs):
    r = _RUN_CACHE["runner"]
    return [
        {name: np.asarray(out_arrs[i]).reshape(r["n_cores"], *r["out_avals"][i].shape)[c]
         for i, name in enumerate(r["out_names"])}
        for c in range(r["n_cores"])
    ]


def _consts():
    import ml_dtypes
    c = {}
    c["onr_d"] = np.ones((1, P), np.float32)
    c["idb_d"] = np.eye(P).astype(ml_dtypes.bfloat16)
    p_ = np.arange(P)[:, None]
    f_ = np.arange(MT)[None, :]
    c["tri_d"] = np.where(p_ <= f_, 0.0, NEG).astype(np.float32)
    return c


def _route_host(hidden_states, router_w):
    """Exact replica of the reference routing, on jax CPU."""
    import jax
    import jax.numpy as jnp
    cpu = jax.devices("cpu")[0]
    with jax.default_device(cpu):
        w = jnp.einsum('bsd,d->bs', jnp.asarray(hidden_states),
                       jnp.asarray(router_w)[:, 0])
        k = MT
        top_vals, top_idx = jax.lax.top_k(w, k)
        sel_idx = jnp.sort(top_idx[:, :M], axis=1)
        return np.asarray(w), np.asarray(sel_idx)


def _make_shared(Wq, Wk, Wv, Wo, W1, W2, ln1_g, ln1_b, ln2_g, ln2_b):
    import ml_dtypes
    bf = lambda a: np.ascontiguousarray(
        np.asarray(a, np.float32).astype(ml_dtypes.bfloat16))
    rep = lambda v: np.ascontiguousarray(
        np.broadcast_to(np.asarray(v, np.float32)[None, :], (P, D)))
    return {
        "wqd": bf(np.asarray(Wq, np.float32) * (1.0 / np.sqrt(HD))),
        "wkd": bf(Wk), "wvd": bf(Wv), "wod": bf(Wo),
        "w1d": bf(W1), "w2d": bf(W2),
        "ln1g": rep(ln1_g), "ln1b": rep(ln1_b),
        "ln2g": rep(ln2_g), "ln2b": rep(ln2_b),
        **_consts(),
    }


def kernel(hidden_states, attention_mask, position_ids, router_w,
           Wq, Wk, Wv, Wo, W1, W2, ln1_g, ln1_b, ln2_g, ln2_b):
    hidden_states = np.ascontiguousarray(np.asarray(hidden_states, np.float32))
    router_w = np.asarray(router_w, np.float32)

    w, sel = _route_host(hidden_states, router_w)          # [B,S], [B,M]
    rw = w[np.arange(B)[:, None], sel]                     # [B,M]

    pos = np.broadcast_to(np.asarray(position_ids, np.int64), (B, S))
    inv = (1.0 / (10000.0 ** (np.arange(0, HD, 2, dtype=np.float32) / HD)))

    nc = _build_nc()

    # Shared (weight/const) params are cached device-resident; invalidate if
    # the caller passed different weight values than the resident copy.
    orig = (Wq, Wk, Wv, Wo, W1, W2, ln1_g, ln1_b, ln2_g, ln2_b)
    prev_refs = _RUN_CACHE.get("raw_refs")
    same = prev_refs is not None and all(
        p is r for p, r in zip(prev_refs, orig))
    if not same:
        raw = [np.asarray(a, np.float32) for a in orig]
        prev = _RUN_CACHE.get("raw_weights")
        same = prev is not None and all(
            p.shape == r.shape and np.array_equal(p, r)
            for p, r in zip(prev, raw))
        if not same:
            _RUN_CACHE["raw_weights"] = [np.array(a, copy=True) for a in raw]
            _RUN_CACHE["shared"] = _make_shared(*raw)
            if "runner" in _RUN_CACHE:
                _RUN_CACHE["runner"]["resident"].clear()
        _RUN_CACHE["raw_refs"] = list(orig)
    shared = _RUN_CACHE["shared"]

    import ml_dtypes
    in_maps = []
    for b in range(B):
        tok = np.zeros((MT, D), ml_dtypes.bfloat16)
        tok[:M] = hidden_states[b, sel[b]].astype(ml_dtypes.bfloat16)
        sel_pos = np.zeros((MT,), np.float32)
        sel_pos[:M] = pos[b, sel[b]].astype(np.float32)
        ang = sel_pos[:, None] * inv[None, :]              # [MT, 32]
        ct = np.cos(ang).astype(np.float32).T              # [32, MT]
        st = np.sin(ang).astype(np.float32).T
        m = {"tok": tok,
             "cosT_d": np.ascontiguousarray(ct),
             "sinT_d": np.ascontiguousarray(st)}
        in_maps.append(m)

    import os
    trace = os.environ.get("BASS_TRACE") and not os.environ.get("BASS_NEVER_TRACE")
    results = None
    if not trace:
        try:
            out_arrs = _run_fast(nc, shared, in_maps)
            out = np.array(hidden_states, copy=True)   # overlaps device exec
            results = _fetch(out_arrs)
        except Exception:
            _RUN_CACHE.pop("runner", None)
            results = None
    if results is None:
        full_maps = [{**shared, **m} for m in in_maps]
        results = run_bass_kernel_spmd(nc, full_maps, core_ids=[0, 1]).results
        out = np.array(hidden_states, copy=True)

    for b in range(B):
        x3 = results[b]["x3o"][:M].astype(np.float32)
        out[b, sel[b]] = x3 * rw[b][:, None]
    return out
